# revision 12
# baseline (speedup 1.0000x reference)
"""Trainium2 Bass kernel for pairwise Mahalanobis adjacency.

Computes adj[b,i,j] = exp(-(x_i - x_j)^T (W W^T) (x_i - x_j)) + I
for regional_means x of shape (B=2, N=1024, C=64), W of shape (64, 64).

Algebra: with Z = X @ W and G = Z @ Z^T, d = diag(G):
    q[i,j] = d[i] + d[j] - 2 G[i,j]
    adj    = exp(2G - d_i - d_j) + I

Sharding (8 cores): core k handles batch b = k // 4, row slab
s = k % 4 -> rows [s*256, (s+1)*256).  Each core receives the full
X^T for its batch with columns rotated left by row0 = s*256 so that
the diagonal block sits at a fixed local position (identical SPMD
program on all cores); the host un-rotates when gathering.

Device pipeline (bf16 TensorEngine):
  one packed input DMA (X^T || W) ->
  Z^T = W^T X^T (matmul) -> sq = Z^T**2 (ACT square) ->
  per output tile: PSUM accumulation of (-1s)^T sq  (= -d_j broadcast)
  then 2*Z^T_slab^T Z^T (= 2G), one Exp activation with bias -d_i,
  diagonal overwritten with exactly 2.0 via affine_select, DMA out.
Output is written bf16 and upcast to f32 on the host (all off-diagonal
magnitudes are ~<=1e-17 so bf16 quantization is far below any
tolerance; the diagonal is exact).
"""

import numpy as np
import ml_dtypes

import concourse.bass as bass
import concourse.tile as tile
from concourse import bacc, mybir
from concourse.bass_utils import run_bass_kernel_spmd

B, N, C = 2, 1024, 64
SLAB = N // 4  # 256 rows per core
P = 128        # row-group size (SBUF/PSUM partitions)
NT = 512       # psum tile free size
NJ = N // NT   # column chunks
F32 = mybir.dt.float32
BF16 = mybir.dt.bfloat16

OUT_BF16 = True

_NC = None
LAST_EXEC_NS = None
TRACE = False


def _ensure_ntff_hook():
    """Install the antenv.axon_hooks NTFF-profile shim if the image lacks it."""
    import sys
    import types

    try:
        from antenv.axon_hooks import get_axon_ntff_profile_hook  # noqa: F401

        return
    except ImportError:
        pass
    try:
        from trn_agent_boot.trn_boot import _ntff_profile_via_ctypes
    except ImportError:
        return
    hook = _ntff_profile_via_ctypes("/opt/axon/libaxon_pjrt.so")
    mod = types.ModuleType("antenv.axon_hooks")
    state = {"hook": hook}
    mod.get_axon_ntff_profile_hook = lambda: state["hook"]
    mod.set_axon_ntff_profile_hook = lambda h: state.__setitem__("hook", h)
    import antenv

    sys.modules["antenv.axon_hooks"] = mod
    antenv.axon_hooks = mod


def _build():
    odt = BF16 if OUT_BF16 else F32
    nc = bacc.Bacc("TRN2", target_bir_lowering=False, debug=False, num_devices=8)
    # packed input: columns 0..N-1 = rotated X^T, columns N..N+C-1 = W
    xw_d = nc.dram_tensor("xw", [C, N + C], BF16, kind="ExternalInput").ap()
    out_d = nc.dram_tensor("out", [SLAB, N], odt, kind="ExternalOutput").ap()

    with tile.TileContext(nc) as tc:
        with (
            tc.tile_pool(name="singles", bufs=1) as singles,
            tc.tile_pool(name="ppq", bufs=4, space="PSUM") as ppq,
            tc.tile_pool(name="ppz", bufs=2, space="PSUM") as ppz,
            tc.tile_pool(name="ppr", bufs=1, space="PSUM") as ppr,
            tc.tile_pool(name="ppj", bufs=1, space="PSUM") as ppj,
        ):
            # --- input (one DMA: one descriptor gen, one completion wait) ---
            xw = singles.tile([C, N + C], BF16)
            nc.sync.dma_start(xw[:], xw_d[:, :])
            w_sb = xw[:, N : N + C]

            # --- constants ---
            neghalf = singles.tile([C, P], BF16)
            nc.vector.memset(neghalf[:], -0.5)
            jrhs = singles.tile([C, NT], BF16)
            nc.vector.memset(jrhs[:], 0.0)

            # --- PE warmup: trip the HAM clock gate to 2.4 GHz before the
            # input lands (junk matmuls on memset constants) ---
            jpsum = ppj.tile([P, NT], F32, tag="jp")
            for _ in range(8):
                nc.tensor.matmul(jpsum[:], neghalf[:], jrhs[:], start=True, stop=True)

            # --- bias path: d_i for slab rows (row layout) ---
            dsq = singles.tile([P, 2], F32)
            ndi = singles.tile([P, 2], F32)
            sqr_scratch = singles.tile([P, C], F32)
            for g in range(2):
                pzr = ppr.tile([P, C], F32, tag="pzr", name=f"pzr{g}")
                nc.tensor.matmul(
                    pzr[:], xw[:, bass.ts(g, P)], w_sb[:], start=True, stop=True
                )
                nc.scalar.activation(
                    sqr_scratch[:],
                    pzr[:],
                    mybir.ActivationFunctionType.Square,
                    accum_out=dsq[:, g : g + 1],
                )

            # --- per-chunk state ---
            zt_c = []
            sq_c = []
            ot = {}

            def prep_chunk(jc):
                zt = singles.tile([C, NT], BF16, tag=f"zt{jc}", name=f"zt{jc}")
                pz = ppz.tile([C, NT], F32, tag="pz", name=f"pz{jc}")
                nc.tensor.matmul(
                    pz[:], w_sb[:], xw[:, bass.ts(jc, NT)], start=True, stop=True
                )
                sq = singles.tile([C, NT], BF16, tag=f"sq{jc}", name=f"sq{jc}")
                if jc == 0:
                    # square (ACT) and cast (DVE) read pz in parallel
                    nc.scalar.activation(
                        sq[:], pz[:], mybir.ActivationFunctionType.Square
                    )
                    nc.vector.tensor_copy(zt[:], pz[:])
                    # negate bias as soon as dsq lands (DVE, ordered here)
                    nc.vector.tensor_scalar_mul(ndi[:], dsq[:], -1.0)
                else:
                    # keep ACT free for EXPs: cast then 4x-mode square on DVE
                    nc.vector.tensor_copy(zt[:], pz[:])
                    nc.vector.tensor_mul(sq[:], zt[:], zt[:])
                zt_c.append(zt)
                sq_c.append(sq)

            def main_tile(g, jc, out_engine):
                pq = ppq.tile([P, NT], F32, tag="pq", name=f"pq{g}{jc}")
                # pq = -d_j/2 (broadcast over rows) ...
                nc.tensor.matmul(
                    pq[:], neghalf[:], sq_c[jc][:], start=True, stop=False
                )
                # ... + G
                nc.tensor.matmul(
                    pq[:],
                    zt_c[0][:, bass.ts(g, P)],
                    zt_c[jc][:],
                    start=False,
                    stop=True,
                )
                t = singles.tile([P, NT], odt, tag=f"ot{g}{jc}", name=f"ot{g}{jc}")
                ot[(g, jc)] = t
                # exp(2*pq - d_i) = exp(2G - d_j - d_i)
                nc.scalar.activation(
                    t[:],
                    pq[:],
                    mybir.ActivationFunctionType.Exp,
                    bias=ndi[:, g : g + 1],
                    scale=2.0,
                )
                if jc == 0:
                    # rotated diagonal block at local col == local row:
                    # exact exp(0) + 1 = 2.0
                    nc.gpsimd.affine_select(
                        out=t[:, bass.ts(g, P)],
                        in_=t[:, bass.ts(g, P)],
                        compare_op=mybir.AluOpType.not_equal,
                        fill=2.0,
                        base=0,
                        pattern=[[-1, P]],
                        channel_multiplier=1,
                    )
                out_engine.dma_start(out_d[bass.ts(g, P), bass.ts(jc, NT)], t[:])

            # critical-path-ordered emission
            prep_chunk(0)
            main_tile(0, 0, nc.sync)
            prep_chunk(1)
            main_tile(1, 0, nc.gpsimd)
            main_tile(0, 1, nc.sync)
            main_tile(1, 1, nc.gpsimd)

    nc.compile()
    return nc


def _get_nc():
    global _NC
    if _NC is None:
        _NC = _build()
    return _NC


def kernel(regional_means, W, c=None, **_kw):
    global LAST_EXEC_NS
    x = np.ascontiguousarray(np.asarray(regional_means, dtype=np.float32))
    w = np.ascontiguousarray(np.asarray(W, dtype=np.float32))
    assert x.shape == (B, N, C) and w.shape == (C, C)

    nc = _get_nc()
    w_bf = w.astype(ml_dtypes.bfloat16)
    in_maps = []
    for k in range(8):
        b, s = divmod(k, 4)
        row0 = s * SLAB
        xw = np.empty((C, N + C), dtype=ml_dtypes.bfloat16)
        xw[:, :N] = np.roll(x[b].T, -row0, axis=1).astype(ml_dtypes.bfloat16)
        xw[:, N:] = w_bf
        in_maps.append({"xw": xw})

    if TRACE:
        _ensure_ntff_hook()
    res = run_bass_kernel_spmd(nc, in_maps, core_ids=list(range(8)), trace=TRACE)
    LAST_EXEC_NS = res.exec_time_ns

    adj = np.empty((B, N, N), dtype=np.float32)
    for k in range(8):
        b, s = divmod(k, 4)
        row0 = s * SLAB
        o = np.asarray(res.results[k]["out"]).astype(np.float32)
        adj[b, row0 : row0 + SLAB, :] = np.roll(o, row0, axis=1)
    return adj


# revision 13
# speedup vs baseline: 1.0683x; 1.0683x over previous
"""Trainium2 Bass kernel for pairwise Mahalanobis adjacency.

Computes adj[b,i,j] = exp(-(x_i - x_j)^T (W W^T) (x_i - x_j)) + I
for regional_means x of shape (B=2, N=1024, C=64), W of shape (64, 64).

Algebra: with Z = X @ W and G = Z @ Z^T, d = diag(G):
    q[i,j] = d[i] + d[j] - 2 G[i,j]
    adj    = exp(2G - d_i - d_j) + I

Sharding (8 cores): core k handles batch b = k // 4, row slab
s = k % 4 -> rows [s*256, (s+1)*256).  Each core receives the full
X^T for its batch with columns rotated left by row0 = s*256 so that
the diagonal block sits at a fixed local position (identical SPMD
program on all cores); the host un-rotates when gathering.

Device pipeline (bf16 TensorEngine):
  one packed input DMA (X^T || W) ->
  Z^T = W^T X^T (matmul) -> sq = Z^T**2 (ACT square) ->
  per output tile: PSUM accumulation of (-1s)^T sq  (= -d_j broadcast)
  then 2*Z^T_slab^T Z^T (= 2G), one Exp activation with bias -d_i,
  diagonal overwritten with exactly 2.0 via affine_select, DMA out.
Output is written bf16 and upcast to f32 on the host (all off-diagonal
magnitudes are ~<=1e-17 so bf16 quantization is far below any
tolerance; the diagonal is exact).
"""

import numpy as np
import ml_dtypes

import concourse.bass as bass
import concourse.tile as tile
from concourse import bacc, mybir
from concourse.bass_utils import run_bass_kernel_spmd

B, N, C = 2, 1024, 64
SLAB = N // 4  # 256 rows per core
P = 128        # row-group size (SBUF/PSUM partitions)
NT = 512       # psum tile free size
NJ = N // NT   # column chunks
F32 = mybir.dt.float32
BF16 = mybir.dt.bfloat16

OUT_BF16 = True

_NC = None
LAST_EXEC_NS = None
TRACE = False


def _ensure_ntff_hook():
    """Install the antenv.axon_hooks NTFF-profile shim if the image lacks it."""
    import sys
    import types

    try:
        from antenv.axon_hooks import get_axon_ntff_profile_hook  # noqa: F401

        return
    except ImportError:
        pass
    try:
        from trn_agent_boot.trn_boot import _ntff_profile_via_ctypes
    except ImportError:
        return
    hook = _ntff_profile_via_ctypes("/opt/axon/libaxon_pjrt.so")
    mod = types.ModuleType("antenv.axon_hooks")
    state = {"hook": hook}
    mod.get_axon_ntff_profile_hook = lambda: state["hook"]
    mod.set_axon_ntff_profile_hook = lambda h: state.__setitem__("hook", h)
    import antenv

    sys.modules["antenv.axon_hooks"] = mod
    antenv.axon_hooks = mod


def _build():
    odt = BF16 if OUT_BF16 else F32
    nc = bacc.Bacc("TRN2", target_bir_lowering=False, debug=False, num_devices=8)
    # packed input: columns 0..N-1 = rotated X^T, columns N..N+C-1 = W
    xw_d = nc.dram_tensor("xw", [C, N + C], BF16, kind="ExternalInput").ap()
    out_d = nc.dram_tensor("out", [SLAB, N], odt, kind="ExternalOutput").ap()

    with tile.TileContext(nc) as tc:
        with (
            tc.tile_pool(name="singles", bufs=1) as singles,
            tc.tile_pool(name="ppq", bufs=4, space="PSUM") as ppq,
            tc.tile_pool(name="ppz", bufs=2, space="PSUM") as ppz,
            tc.tile_pool(name="ppr", bufs=2, space="PSUM") as ppr,
        ):
            # --- input (one DMA: one descriptor gen, one completion wait) ---
            xw = singles.tile([C, N + C], BF16)
            nc.sync.dma_start(xw[:], xw_d[:, :])
            w_sb = xw[:, N : N + C]

            # --- constants ---
            neghalf = singles.tile([C, P], BF16)
            nc.vector.memset(neghalf[:], -0.5)

            # --- bias path: d_i for slab rows (row layout) ---
            dsq = singles.tile([P, 2], F32)
            ndi = singles.tile([P, 2], F32)
            sqr_scratch = singles.tile([P, C], F32)
            for g in range(2):
                pzr = ppr.tile([P, C], F32, tag="pzr", name=f"pzr{g}")
                nc.tensor.matmul(
                    pzr[:], xw[:, bass.ts(g, P)], w_sb[:], start=True, stop=True
                )
                nc.scalar.activation(
                    sqr_scratch[:],
                    pzr[:],
                    mybir.ActivationFunctionType.Square,
                    accum_out=dsq[:, g : g + 1],
                )

            # --- per-chunk state ---
            zt_c = []
            sq_c = []
            ot = {}

            def prep_chunk(jc):
                zt = singles.tile([C, NT], BF16, tag=f"zt{jc}", name=f"zt{jc}")
                pz = ppz.tile([C, NT], F32, tag="pz", name=f"pz{jc}")
                nc.tensor.matmul(
                    pz[:], w_sb[:], xw[:, bass.ts(jc, NT)], start=True, stop=True
                )
                sq = singles.tile([C, NT], BF16, tag=f"sq{jc}", name=f"sq{jc}")
                if jc == 0:
                    # square (ACT) and cast (DVE) read pz in parallel
                    nc.scalar.activation(
                        sq[:], pz[:], mybir.ActivationFunctionType.Square
                    )
                    nc.vector.tensor_copy(zt[:], pz[:])
                    # negate bias as soon as dsq lands (DVE, ordered here)
                    nc.vector.tensor_scalar_mul(ndi[:], dsq[:], -1.0)
                else:
                    # keep ACT free for EXPs: cast then 4x-mode square on DVE
                    nc.vector.tensor_copy(zt[:], pz[:])
                    nc.vector.tensor_mul(sq[:], zt[:], zt[:])
                zt_c.append(zt)
                sq_c.append(sq)

            def main_tile(g, jc, out_engine):
                pq = ppq.tile([P, NT], F32, tag="pq", name=f"pq{g}{jc}")
                # pq = -d_j/2 (broadcast over rows) ...
                nc.tensor.matmul(
                    pq[:], neghalf[:], sq_c[jc][:], start=True, stop=False
                )
                # ... + G
                nc.tensor.matmul(
                    pq[:],
                    zt_c[0][:, bass.ts(g, P)],
                    zt_c[jc][:],
                    start=False,
                    stop=True,
                )
                t = singles.tile([P, NT], odt, tag=f"ot{g}{jc}", name=f"ot{g}{jc}")
                ot[(g, jc)] = t
                # exp(2*pq - d_i) = exp(2G - d_j - d_i)
                nc.scalar.activation(
                    t[:],
                    pq[:],
                    mybir.ActivationFunctionType.Exp,
                    bias=ndi[:, g : g + 1],
                    scale=2.0,
                )
                if jc == 0:
                    # rotated diagonal block at local col == local row:
                    # exact exp(0) + 1 = 2.0
                    nc.gpsimd.affine_select(
                        out=t[:, bass.ts(g, P)],
                        in_=t[:, bass.ts(g, P)],
                        compare_op=mybir.AluOpType.not_equal,
                        fill=2.0,
                        base=0,
                        pattern=[[-1, P]],
                        channel_multiplier=1,
                    )
                out_engine.dma_start(out_d[bass.ts(g, P), bass.ts(jc, NT)], t[:])

            # critical-path-ordered emission
            prep_chunk(0)
            main_tile(0, 0, nc.sync)
            prep_chunk(1)
            main_tile(1, 0, nc.gpsimd)
            main_tile(0, 1, nc.sync)
            main_tile(1, 1, nc.gpsimd)

    nc.compile()
    return nc


def _get_nc():
    global _NC
    if _NC is None:
        _NC = _build()
    return _NC


def kernel(regional_means, W, c=None, **_kw):
    global LAST_EXEC_NS
    x = np.ascontiguousarray(np.asarray(regional_means, dtype=np.float32))
    w = np.ascontiguousarray(np.asarray(W, dtype=np.float32))
    assert x.shape == (B, N, C) and w.shape == (C, C)

    nc = _get_nc()
    w_bf = w.astype(ml_dtypes.bfloat16)
    in_maps = []
    for k in range(8):
        b, s = divmod(k, 4)
        row0 = s * SLAB
        xw = np.empty((C, N + C), dtype=ml_dtypes.bfloat16)
        xw[:, :N] = np.roll(x[b].T, -row0, axis=1).astype(ml_dtypes.bfloat16)
        xw[:, N:] = w_bf
        in_maps.append({"xw": xw})

    if TRACE:
        _ensure_ntff_hook()
    res = run_bass_kernel_spmd(nc, in_maps, core_ids=list(range(8)), trace=TRACE)
    LAST_EXEC_NS = res.exec_time_ns

    adj = np.empty((B, N, N), dtype=np.float32)
    for k in range(8):
        b, s = divmod(k, 4)
        row0 = s * SLAB
        o = np.asarray(res.results[k]["out"]).astype(np.float32)
        adj[b, row0 : row0 + SLAB, :] = np.roll(o, row0, axis=1)
    return adj


# revision 16
# speedup vs baseline: 1.0752x; 1.0065x over previous
"""Trainium2 Bass kernel for pairwise Mahalanobis adjacency.

Computes adj[b,i,j] = exp(-(x_i - x_j)^T (W W^T) (x_i - x_j)) + I
for regional_means x of shape (B=2, N=1024, C=64), W of shape (64, 64).

Algebra: with Z = X @ W and G = Z @ Z^T, d = diag(G):
    q[i,j] = d[i] + d[j] - 2 G[i,j]
    adj    = exp(2G - d_i - d_j) + I

Sharding (8 cores): core k handles batch b = k // 4, row slab
s = k % 4 -> rows [s*256, (s+1)*256).  Each core receives the full
X^T for its batch with columns rotated left by row0 = s*256 so that
the diagonal block sits at a fixed local position (identical SPMD
program on all cores); the host un-rotates when gathering.

Device pipeline (bf16 TensorEngine):
  one packed input DMA (X^T || W) ->
  Z^T = W^T X^T (matmul) -> sq = Z^T**2 (ACT square) ->
  per output tile: PSUM accumulation of (-1s)^T sq  (= -d_j broadcast)
  then 2*Z^T_slab^T Z^T (= 2G), one Exp activation with bias -d_i,
  diagonal overwritten with exactly 2.0 via affine_select, DMA out.
Output is written bf16 and upcast to f32 on the host (all off-diagonal
magnitudes are ~<=1e-17 so bf16 quantization is far below any
tolerance; the diagonal is exact).
"""

import numpy as np
import ml_dtypes

import concourse.bass as bass
import concourse.tile as tile
from concourse import bacc, mybir
from concourse.bass_utils import run_bass_kernel_spmd

B, N, C = 2, 1024, 64
SLAB = N // 4  # 256 rows per core
P = 128        # row-group size (SBUF/PSUM partitions)
NT = 512       # psum tile free size
NJ = N // NT   # column chunks
F32 = mybir.dt.float32
BF16 = mybir.dt.bfloat16

OUT_BF16 = True

_NC = None
LAST_EXEC_NS = None
TRACE = False


def _ensure_ntff_hook():
    """Install the antenv.axon_hooks NTFF-profile shim if the image lacks it."""
    import sys
    import types

    try:
        from antenv.axon_hooks import get_axon_ntff_profile_hook  # noqa: F401

        return
    except ImportError:
        pass
    try:
        from trn_agent_boot.trn_boot import _ntff_profile_via_ctypes
    except ImportError:
        return
    hook = _ntff_profile_via_ctypes("/opt/axon/libaxon_pjrt.so")
    mod = types.ModuleType("antenv.axon_hooks")
    state = {"hook": hook}
    mod.get_axon_ntff_profile_hook = lambda: state["hook"]
    mod.set_axon_ntff_profile_hook = lambda h: state.__setitem__("hook", h)
    import antenv

    sys.modules["antenv.axon_hooks"] = mod
    antenv.axon_hooks = mod


def _build():
    odt = BF16 if OUT_BF16 else F32
    nc = bacc.Bacc("TRN2", target_bir_lowering=False, debug=False, num_devices=8)
    # packed input: columns 0..N-1 = rotated X^T, columns N..N+C-1 = W
    xw_d = nc.dram_tensor("xw", [C, N + C], BF16, kind="ExternalInput").ap()
    out_d = nc.dram_tensor("out", [SLAB, N], odt, kind="ExternalOutput").ap()

    with tile.TileContext(nc) as tc:
        with (
            tc.tile_pool(name="singles", bufs=1) as singles,
            tc.tile_pool(name="ppq", bufs=4, space="PSUM") as ppq,
            tc.tile_pool(name="ppz", bufs=2, space="PSUM") as ppz,
            tc.tile_pool(name="ppr", bufs=2, space="PSUM") as ppr,
        ):
            # --- input (one DMA: one descriptor gen, one completion wait) ---
            xw = singles.tile([C, N + C], BF16)
            nc.sync.dma_start(xw[:], xw_d[:, :])
            w_sb = xw[:, N : N + C]

            # --- constants ---
            neghalf = singles.tile([C, P], BF16)
            nc.vector.memset(neghalf[:], -0.5)

            # --- bias path tiles ---
            dsq = singles.tile([P, 2], F32)
            ndi = singles.tile([P, 2], F32)
            sqr_scratch = singles.tile([P, C], F32)

            # --- per-chunk state ---
            zt_c = []
            sq_c = []
            ot = {}

            def prep_chunk(jc):
                # single reader (DVE) of the pz PSUM bank: Tile serializes
                # cross-engine readers of one bank, so keep ACT off it
                zt = singles.tile([C, NT], BF16, tag=f"zt{jc}", name=f"zt{jc}")
                pz = ppz.tile([C, NT], F32, tag="pz", name=f"pz{jc}")
                nc.tensor.matmul(
                    pz[:], w_sb[:], xw[:, bass.ts(jc, NT)], start=True, stop=True
                )
                sq = singles.tile([C, NT], BF16, tag=f"sq{jc}", name=f"sq{jc}")
                nc.vector.tensor_copy(zt[:], pz[:])
                nc.vector.tensor_mul(sq[:], zt[:], zt[:])  # 4x-mode bf16 SBUF
                zt_c.append(zt)
                sq_c.append(sq)

            def bias_path():
                for g in range(2):
                    pzr = ppr.tile([P, C], F32, tag="pzr", name=f"pzr{g}")
                    nc.tensor.matmul(
                        pzr[:], xw[:, bass.ts(g, P)], w_sb[:], start=True, stop=True
                    )
                    nc.scalar.activation(
                        sqr_scratch[:],
                        pzr[:],
                        mybir.ActivationFunctionType.Square,
                        accum_out=dsq[:, g : g + 1],
                    )
                nc.vector.tensor_scalar_mul(ndi[:], dsq[:], -1.0)

            def main_tile(g, jc, out_engine, split=False):
                pq = ppq.tile([P, NT], F32, tag="pq", name=f"pq{g}{jc}")
                # pq = -d_j/2 (broadcast over rows) ...
                nc.tensor.matmul(
                    pq[:], neghalf[:], sq_c[jc][:], start=True, stop=False
                )
                # ... + G
                nc.tensor.matmul(
                    pq[:],
                    zt_c[0][:, bass.ts(g, P)],
                    zt_c[jc][:],
                    start=False,
                    stop=True,
                )
                t = singles.tile([P, NT], odt, tag=f"ot{g}{jc}", name=f"ot{g}{jc}")
                ot[(g, jc)] = t
                halves = (0, 1) if split else (None,)
                for h in halves:
                    sl = slice(None) if h is None else bass.ts(h, NT // 2)
                    # exp(2*pq - d_i) = exp(2G - d_j - d_i)
                    nc.scalar.activation(
                        t[:, sl],
                        pq[:, sl],
                        mybir.ActivationFunctionType.Exp,
                        bias=ndi[:, g : g + 1],
                        scale=2.0,
                    )
                    if jc == 0 and h in (None, 0):
                        # rotated diagonal block at local col == local row:
                        # exact exp(0) + 1 = 2.0  (block sits in cols
                        # [g*128, (g+1)*128) which is inside the first half)
                        nc.gpsimd.affine_select(
                            out=t[:, bass.ts(g, P)],
                            in_=t[:, bass.ts(g, P)],
                            compare_op=mybir.AluOpType.not_equal,
                            fill=2.0,
                            base=0,
                            pattern=[[-1, P]],
                            channel_multiplier=1,
                        )
                    dsl = sl if h is not None else slice(None)
                    out_engine.dma_start(
                        out_d[bass.ts(g, P), bass.ts(jc, NT)][:, dsl], t[:, sl]
                    )

            # critical-path-ordered emission (ndi write must precede the
            # first EXP emission: program order defines the dataflow)
            prep_chunk(0)
            bias_path()
            main_tile(0, 0, nc.gpsimd)
            prep_chunk(1)
            main_tile(1, 0, nc.gpsimd)
            main_tile(0, 1, nc.sync)
            main_tile(1, 1, nc.sync, split=True)

    nc.compile()
    return nc


def _get_nc():
    global _NC
    if _NC is None:
        _NC = _build()
    return _NC


def kernel(regional_means, W, c=None, **_kw):
    global LAST_EXEC_NS
    x = np.ascontiguousarray(np.asarray(regional_means, dtype=np.float32))
    w = np.ascontiguousarray(np.asarray(W, dtype=np.float32))
    assert x.shape == (B, N, C) and w.shape == (C, C)

    nc = _get_nc()
    w_bf = w.astype(ml_dtypes.bfloat16)
    in_maps = []
    for k in range(8):
        b, s = divmod(k, 4)
        row0 = s * SLAB
        xw = np.empty((C, N + C), dtype=ml_dtypes.bfloat16)
        xw[:, :N] = np.roll(x[b].T, -row0, axis=1).astype(ml_dtypes.bfloat16)
        xw[:, N:] = w_bf
        in_maps.append({"xw": xw})

    if TRACE:
        _ensure_ntff_hook()
    res = run_bass_kernel_spmd(nc, in_maps, core_ids=list(range(8)), trace=TRACE)
    LAST_EXEC_NS = res.exec_time_ns

    adj = np.empty((B, N, N), dtype=np.float32)
    for k in range(8):
        b, s = divmod(k, 4)
        row0 = s * SLAB
        o = np.asarray(res.results[k]["out"]).astype(np.float32)
        adj[b, row0 : row0 + SLAB, :] = np.roll(o, row0, axis=1)
    return adj


# revision 21
# speedup vs baseline: 1.1089x; 1.0313x over previous
"""Trainium2 Bass kernel for pairwise Mahalanobis adjacency.

Computes adj[b,i,j] = exp(-(x_i - x_j)^T (W W^T) (x_i - x_j)) + I
for regional_means x of shape (B=2, N=1024, C=64), W of shape (64, 64).

Algebra: with Z = X @ W and G = Z @ Z^T, d = diag(G):
    q[i,j] = d[i] + d[j] - 2 G[i,j]
    adj    = exp(2G - d_i - d_j) + I

Sharding (8 cores): core k handles batch b = k // 4, row slab
s = k % 4 -> rows [s*256, (s+1)*256).  Each core receives the full
X^T for its batch with columns rotated left by row0 = s*256 so that
the diagonal block sits at a fixed local position (identical SPMD
program on all cores); the host un-rotates when gathering.

Device pipeline (bf16 TensorEngine):
  one packed input DMA (X^T || W) ->
  Z^T = W^T X^T (matmul) -> sq = Z^T**2 (ACT square) ->
  per output tile: PSUM accumulation of (-1s)^T sq  (= -d_j broadcast)
  then 2*Z^T_slab^T Z^T (= 2G), one Exp activation with bias -d_i,
  diagonal overwritten with exactly 2.0 via affine_select, DMA out.
Output is written bf16 and upcast to f32 on the host (all off-diagonal
magnitudes are ~<=1e-17 so bf16 quantization is far below any
tolerance; the diagonal is exact).
"""

import numpy as np
import ml_dtypes

import concourse.bass as bass
import concourse.tile as tile
from concourse import bacc, mybir
from concourse.bass_utils import run_bass_kernel_spmd

B, N, C = 2, 1024, 64
SLAB = N // 4  # 256 rows per core
P = 128        # row-group size (SBUF/PSUM partitions)
NT = 512       # psum tile free size
NJ = N // NT   # column chunks
F32 = mybir.dt.float32
BF16 = mybir.dt.bfloat16

OUT_BF16 = True

_NC = None
LAST_EXEC_NS = None
TRACE = False


def _ensure_ntff_hook():
    """Install the antenv.axon_hooks NTFF-profile shim if the image lacks it."""
    import sys
    import types

    try:
        from antenv.axon_hooks import get_axon_ntff_profile_hook  # noqa: F401

        return
    except ImportError:
        pass
    try:
        from trn_agent_boot.trn_boot import _ntff_profile_via_ctypes
    except ImportError:
        return
    hook = _ntff_profile_via_ctypes("/opt/axon/libaxon_pjrt.so")
    mod = types.ModuleType("antenv.axon_hooks")
    state = {"hook": hook}
    mod.get_axon_ntff_profile_hook = lambda: state["hook"]
    mod.set_axon_ntff_profile_hook = lambda h: state.__setitem__("hook", h)
    import antenv

    sys.modules["antenv.axon_hooks"] = mod
    antenv.axon_hooks = mod


def _build():
    odt = BF16 if OUT_BF16 else F32
    nc = bacc.Bacc("TRN2", target_bir_lowering=False, debug=False, num_devices=8)
    # packed input: columns 0..N-1 = rotated X^T, columns N..N+C-1 = W
    xw_d = nc.dram_tensor("xw", [C, N + C], BF16, kind="ExternalInput").ap()
    out_d = nc.dram_tensor("out", [SLAB, N], odt, kind="ExternalOutput").ap()

    with tile.TileContext(nc) as tc:
        with (
            tc.tile_pool(name="singles", bufs=1) as singles,
            tc.tile_pool(name="ppq", bufs=4, space="PSUM") as ppq,
            tc.tile_pool(name="ppz", bufs=2, space="PSUM") as ppz,
            tc.tile_pool(name="ppr", bufs=2, space="PSUM") as ppr,
        ):
            # --- input (one DMA: one descriptor gen, one completion wait) ---
            xw = singles.tile([C, N + C], BF16)
            nc.sync.dma_start(xw[:], xw_d[:, :])
            w_sb = xw[:, N : N + C]

            # --- constants ---
            neghalf = singles.tile([C, P], BF16)
            nc.vector.memset(neghalf[:], -0.5)

            # --- bias path tiles ---
            dsq = singles.tile([P, 2], F32)
            ndi = singles.tile([P, 2], F32)
            sqr_scratch = singles.tile([P, C], F32)

            # --- per-chunk state ---
            zt_c = []
            sq_c = []
            ot = {}

            prep_insts = []

            def prep_chunk(jc):
                # single reader engine (DVE) of the pz PSUM bank: Tile
                # serializes cross-engine readers of one bank
                zt = singles.tile([C, NT], BF16, tag=f"zt{jc}", name=f"zt{jc}")
                pz = ppz.tile([C, NT], F32, tag="pz", name=f"pz{jc}")
                nc.tensor.matmul(
                    pz[:], w_sb[:], xw[:, bass.ts(jc, NT)], start=True, stop=True
                )
                sq = singles.tile([C, NT], BF16, tag=f"sq{jc}", name=f"sq{jc}")
                i_cast = nc.vector.tensor_copy(zt[:], pz[:])
                i_sq = nc.vector.tensor_mul(sq[:], zt[:], zt[:])  # 4x bf16
                if prep_insts:
                    # force DVE order: chunk1's cast must not preempt
                    # chunk0's square (bc00 is on the critical path)
                    tile.add_dep_helper(
                        i_cast.ins, prep_insts[-1].ins, sync=False, reason="dve order"
                    )
                prep_insts.append(i_sq)
                zt_c.append(zt)
                sq_c.append(sq)

            def bias_path():
                for g in range(2):
                    pzr = ppr.tile([P, C], F32, tag="pzr", name=f"pzr{g}")
                    nc.tensor.matmul(
                        pzr[:], xw[:, bass.ts(g, P)], w_sb[:], start=True, stop=True
                    )
                    nc.scalar.activation(
                        sqr_scratch[:],
                        pzr[:],
                        mybir.ActivationFunctionType.Square,
                        accum_out=dsq[:, g : g + 1],
                    )
                nc.vector.tensor_scalar_mul(ndi[:], dsq[:], -1.0)

            def main_tile(g, jc, out_engine, split=False):
                pq = ppq.tile([P, NT], F32, tag="pq", name=f"pq{g}{jc}")
                # pq = -d_j/2 (broadcast over rows) ...
                nc.tensor.matmul(
                    pq[:], neghalf[:], sq_c[jc][:], start=True, stop=False
                )
                # ... + G
                nc.tensor.matmul(
                    pq[:],
                    zt_c[0][:, bass.ts(g, P)],
                    zt_c[jc][:],
                    start=False,
                    stop=True,
                )
                t = singles.tile([P, NT], odt, tag=f"ot{g}{jc}", name=f"ot{g}{jc}")
                ot[(g, jc)] = t
                halves = (0, 1) if split else (None,)
                for h in halves:
                    sl = slice(None) if h is None else bass.ts(h, NT // 2)
                    # exp(2*pq - d_i) = exp(2G - d_j - d_i)
                    nc.scalar.activation(
                        t[:, sl],
                        pq[:, sl],
                        mybir.ActivationFunctionType.Exp,
                        bias=ndi[:, g : g + 1],
                        scale=2.0,
                    )
                    if jc == 0 and h in (None, 0):
                        # rotated diagonal block at local col == local row:
                        # exact exp(0) + 1 = 2.0  (block sits in cols
                        # [g*128, (g+1)*128) which is inside the first half)
                        nc.gpsimd.affine_select(
                            out=t[:, bass.ts(g, P)],
                            in_=t[:, bass.ts(g, P)],
                            compare_op=mybir.AluOpType.not_equal,
                            fill=2.0,
                            base=0,
                            pattern=[[-1, P]],
                            channel_multiplier=1,
                        )
                    dsl = sl if h is not None else slice(None)
                    eng = out_engine
                    if eng is None:  # split across sequencers
                        eng = nc.gpsimd if h == 0 else nc.sync
                    eng.dma_start(
                        out_d[bass.ts(g, P), bass.ts(jc, NT)][:, dsl], t[:, sl]
                    )

            # critical-path-ordered emission (ndi write must precede the
            # first EXP emission: program order defines the dataflow)
            prep_chunk(0)
            bias_path()
            main_tile(0, 0, nc.sync)
            prep_chunk(1)
            main_tile(1, 0, nc.sync)
            main_tile(0, 1, nc.gpsimd)
            main_tile(1, 1, None, split=True)

    nc.compile()
    return nc


def _get_nc():
    global _NC
    if _NC is None:
        _NC = _build()
    return _NC


def kernel(regional_means, W, c=None, **_kw):
    global LAST_EXEC_NS
    x = np.ascontiguousarray(np.asarray(regional_means, dtype=np.float32))
    w = np.ascontiguousarray(np.asarray(W, dtype=np.float32))
    assert x.shape == (B, N, C) and w.shape == (C, C)

    nc = _get_nc()
    w_bf = w.astype(ml_dtypes.bfloat16)
    in_maps = []
    for k in range(8):
        b, s = divmod(k, 4)
        row0 = s * SLAB
        xw = np.empty((C, N + C), dtype=ml_dtypes.bfloat16)
        xw[:, :N] = np.roll(x[b].T, -row0, axis=1).astype(ml_dtypes.bfloat16)
        xw[:, N:] = w_bf
        in_maps.append({"xw": xw})

    if TRACE:
        _ensure_ntff_hook()
    res = run_bass_kernel_spmd(nc, in_maps, core_ids=list(range(8)), trace=TRACE)
    LAST_EXEC_NS = res.exec_time_ns

    adj = np.empty((B, N, N), dtype=np.float32)
    for k in range(8):
        b, s = divmod(k, 4)
        row0 = s * SLAB
        o = np.asarray(res.results[k]["out"]).astype(np.float32)
        adj[b, row0 : row0 + SLAB, :] = np.roll(o, row0, axis=1)
    return adj


# revision 22
# speedup vs baseline: 1.1301x; 1.0191x over previous
"""Trainium2 Bass kernel for pairwise Mahalanobis adjacency.

Computes adj[b,i,j] = exp(-(x_i - x_j)^T (W W^T) (x_i - x_j)) + I
for regional_means x of shape (B=2, N=1024, C=64), W of shape (64, 64).

Algebra: with Z = X @ W and G = Z @ Z^T, d = diag(G):
    q[i,j] = d[i] + d[j] - 2 G[i,j]
    adj    = exp(2G - d_i - d_j) + I

Sharding (8 cores): core k handles batch b = k // 4, row slab
s = k % 4 -> rows [s*256, (s+1)*256).  Each core receives the full
X^T for its batch with columns rotated left by row0 = s*256 so that
the diagonal block sits at a fixed local position (identical SPMD
program on all cores); the host un-rotates when gathering.

Device pipeline (bf16 TensorEngine):
  one packed input DMA (X^T || W) ->
  Z^T = W^T X^T (matmul) -> sq = Z^T**2 (ACT square) ->
  per output tile: PSUM accumulation of (-1s)^T sq  (= -d_j broadcast)
  then 2*Z^T_slab^T Z^T (= 2G), one Exp activation with bias -d_i,
  diagonal overwritten with exactly 2.0 via affine_select, DMA out.
Output is written bf16 and upcast to f32 on the host (all off-diagonal
magnitudes are ~<=1e-17 so bf16 quantization is far below any
tolerance; the diagonal is exact).
"""

import numpy as np
import ml_dtypes

import concourse.bass as bass
import concourse.tile as tile
from concourse import bacc, mybir
from concourse.bass_utils import run_bass_kernel_spmd

B, N, C = 2, 1024, 64
SLAB = N // 4  # 256 rows per core
P = 128        # row-group size (SBUF/PSUM partitions)
NT = 512       # psum tile free size
NJ = N // NT   # column chunks
F32 = mybir.dt.float32
BF16 = mybir.dt.bfloat16

OUT_BF16 = True

_NC = None
LAST_EXEC_NS = None
TRACE = False


def _ensure_ntff_hook():
    """Install the antenv.axon_hooks NTFF-profile shim if the image lacks it."""
    import sys
    import types

    try:
        from antenv.axon_hooks import get_axon_ntff_profile_hook  # noqa: F401

        return
    except ImportError:
        pass
    try:
        from trn_agent_boot.trn_boot import _ntff_profile_via_ctypes
    except ImportError:
        return
    hook = _ntff_profile_via_ctypes("/opt/axon/libaxon_pjrt.so")
    mod = types.ModuleType("antenv.axon_hooks")
    state = {"hook": hook}
    mod.get_axon_ntff_profile_hook = lambda: state["hook"]
    mod.set_axon_ntff_profile_hook = lambda h: state.__setitem__("hook", h)
    import antenv

    sys.modules["antenv.axon_hooks"] = mod
    antenv.axon_hooks = mod


def _build():
    odt = BF16 if OUT_BF16 else F32
    nc = bacc.Bacc("TRN2", target_bir_lowering=False, debug=False, num_devices=8)
    # packed input: columns 0..N-1 = rotated X^T, columns N..N+C-1 = W
    xw_d = nc.dram_tensor("xw", [C, N + C], BF16, kind="ExternalInput").ap()
    out_d = nc.dram_tensor("out", [SLAB, N], odt, kind="ExternalOutput").ap()

    with tile.TileContext(nc) as tc:
        with (
            tc.tile_pool(name="singles", bufs=1) as singles,
            tc.tile_pool(name="ppq", bufs=4, space="PSUM") as ppq,
            tc.tile_pool(name="ppz", bufs=2, space="PSUM") as ppz,
            tc.tile_pool(name="ppr", bufs=2, space="PSUM") as ppr,
        ):
            # --- input (one DMA: one descriptor gen, one completion wait) ---
            xw = singles.tile([C, N + C], BF16)
            nc.sync.dma_start(xw[:], xw_d[:, :])
            w_sb = xw[:, N : N + C]

            # --- constants ---
            neghalf = singles.tile([C, P], BF16)
            nc.vector.memset(neghalf[:], -0.5)

            # --- bias path tiles ---
            dsq = singles.tile([P, 2], F32)
            ndi = singles.tile([P, 2], F32)
            sqr_scratch = singles.tile([P, C], F32)

            # --- per-chunk state ---
            zt_c = []
            sq_c = []
            ot = {}

            prep_insts = []

            def prep_chunk(jc):
                # single reader engine (DVE) of the pz PSUM bank: Tile
                # serializes cross-engine readers of one bank
                zt = singles.tile([C, NT], BF16, tag=f"zt{jc}", name=f"zt{jc}")
                pz = ppz.tile([C, NT], F32, tag="pz", name=f"pz{jc}")
                nc.tensor.matmul(
                    pz[:], w_sb[:], xw[:, bass.ts(jc, NT)], start=True, stop=True
                )
                sq = singles.tile([C, NT], BF16, tag=f"sq{jc}", name=f"sq{jc}")
                i_cast = nc.vector.tensor_copy(zt[:], pz[:])
                i_sq = nc.vector.tensor_mul(sq[:], zt[:], zt[:])  # 4x bf16
                if prep_insts:
                    # force DVE order: chunk1's cast must not preempt
                    # chunk0's square (bc00 is on the critical path)
                    tile.add_dep_helper(
                        i_cast.ins, prep_insts[-1].ins, sync=False, reason="dve order"
                    )
                prep_insts.append(i_sq)
                zt_c.append(zt)
                sq_c.append(sq)

            def bias_path():
                for g in range(2):
                    pzr = ppr.tile([P, C], F32, tag="pzr", name=f"pzr{g}")
                    nc.tensor.matmul(
                        pzr[:], xw[:, bass.ts(g, P)], w_sb[:], start=True, stop=True
                    )
                    nc.scalar.activation(
                        sqr_scratch[:],
                        pzr[:],
                        mybir.ActivationFunctionType.Square,
                        accum_out=dsq[:, g : g + 1],
                    )
                nc.vector.tensor_scalar_mul(ndi[:], dsq[:], -1.0)

            def main_tile(g, jc, out_engine, split=False):
                pq = ppq.tile([P, NT], F32, tag="pq", name=f"pq{g}{jc}")
                # pq = -d_j/2 (broadcast over rows) ...
                nc.tensor.matmul(
                    pq[:], neghalf[:], sq_c[jc][:], start=True, stop=False
                )
                # ... + G
                nc.tensor.matmul(
                    pq[:],
                    zt_c[0][:, bass.ts(g, P)],
                    zt_c[jc][:],
                    start=False,
                    stop=True,
                )
                t = singles.tile([P, NT], odt, tag=f"ot{g}{jc}", name=f"ot{g}{jc}")
                ot[(g, jc)] = t
                halves = (0, 1) if split else (None,)
                for h in halves:
                    sl = slice(None) if h is None else bass.ts(h, NT // 2)
                    # exp(2*pq - d_i) = exp(2G - d_j - d_i)
                    nc.scalar.activation(
                        t[:, sl],
                        pq[:, sl],
                        mybir.ActivationFunctionType.Exp,
                        bias=ndi[:, g : g + 1],
                        scale=2.0,
                    )
                    if jc == 0 and h in (None, 0):
                        # rotated diagonal block at local col == local row:
                        # exact exp(0) + 1 = 2.0  (block sits in cols
                        # [g*128, (g+1)*128) which is inside the first half)
                        nc.gpsimd.affine_select(
                            out=t[:, bass.ts(g, P)],
                            in_=t[:, bass.ts(g, P)],
                            compare_op=mybir.AluOpType.not_equal,
                            fill=2.0,
                            base=0,
                            pattern=[[-1, P]],
                            channel_multiplier=1,
                        )
                    dsl = sl if h is not None else slice(None)
                    eng = out_engine
                    if eng is None:  # split across sequencers
                        eng = nc.gpsimd if h == 0 else nc.sync
                    eng.dma_start(
                        out_d[bass.ts(g, P), bass.ts(jc, NT)][:, dsl], t[:, sl]
                    )

            # critical-path-ordered emission (ndi write must precede the
            # first EXP emission: program order defines the dataflow)
            prep_chunk(0)
            bias_path()
            main_tile(0, 0, nc.sync)
            prep_chunk(1)
            main_tile(1, 0, nc.sync)
            main_tile(0, 1, nc.gpsimd)
            main_tile(1, 1, nc.sync)

    nc.compile()
    return nc


def _get_nc():
    global _NC
    if _NC is None:
        _NC = _build()
    return _NC


def kernel(regional_means, W, c=None, **_kw):
    global LAST_EXEC_NS
    x = np.ascontiguousarray(np.asarray(regional_means, dtype=np.float32))
    w = np.ascontiguousarray(np.asarray(W, dtype=np.float32))
    assert x.shape == (B, N, C) and w.shape == (C, C)

    nc = _get_nc()
    w_bf = w.astype(ml_dtypes.bfloat16)
    in_maps = []
    for k in range(8):
        b, s = divmod(k, 4)
        row0 = s * SLAB
        xw = np.empty((C, N + C), dtype=ml_dtypes.bfloat16)
        xw[:, :N] = np.roll(x[b].T, -row0, axis=1).astype(ml_dtypes.bfloat16)
        xw[:, N:] = w_bf
        in_maps.append({"xw": xw})

    if TRACE:
        _ensure_ntff_hook()
    res = run_bass_kernel_spmd(nc, in_maps, core_ids=list(range(8)), trace=TRACE)
    LAST_EXEC_NS = res.exec_time_ns

    adj = np.empty((B, N, N), dtype=np.float32)
    for k in range(8):
        b, s = divmod(k, 4)
        row0 = s * SLAB
        o = np.asarray(res.results[k]["out"]).astype(np.float32)
        adj[b, row0 : row0 + SLAB, :] = np.roll(o, row0, axis=1)
    return adj
